# revision 11
# baseline (speedup 1.0000x reference)
"""Trainium2 Bass kernel for nn_Attention (LN + QKV + 8-head attention + out-proj).

Sharding: data-parallel over the 16 (b,p) groups -> 2 groups per core, weights
replicated, no collectives.

Per-core dataflow (matmul compute bf16, f32 accumulation):
  LN per i-tile (bn_stats/bn_aggr, batched ln/exp rstd) -> xhat bf16
  PE-transpose -> xhatT [d,i]
  QKV: qkT [e,i] chunks (w1 stationary); V [j,c] natural (+ ones column so
       the PV matmul accumulates the softmax denominator in row 64)
  Attention per head: ST = kT.T @ qT (K=64, row-paired tile_position);
       pt = exp(ST/8) on ACT (the pacing stream, 128 x ~1.1us);
       aot[0:65] = V.T @ pt accumulated over j-chunks in PSUM
  Per-head normalization entirely off the ACT engine: raw copy to SBUF
       (frees the PSUM bank early) -> DVE reciprocal_approx_fast on the
       denominator row -> gpsimd partition_broadcast -> fused DVE multiply
  final [i,d] = aoT-chunks.T @ w2, DMA out.

Scheduling: w1 columns host-permuted to [q0|k0|v|rest] so heads 0/1 start
~8us after the x DMA; group-1 x prefetched mid-stream; emission order
interleaves qkT chunk-pairs with the attention heads that need them.
"""
import sys
import os

sys.path.insert(0, "/opt/trn_rl_repo")

import numpy as np
import ml_dtypes
from contextlib import ExitStack

import concourse.bass as bass
import concourse.bacc as bacc

# Steer Bacc's activation-table selection to the set containing BOTH exp and
# ln ("natural_log_exp_and_others") so the kernel runs with zero mid-stream
# ACT_TABLE_LOAD swaps.
if not getattr(bacc, "_act_tbl_patched", False):
    _orig_gat = bacc.get_activation_tables

    def _gat_one_set(arch):
        tables = {k: set(v) for k, v in _orig_gat(arch).items()}
        AFT = mybir.ActivationFunctionType
        for name, funcs in tables.items():
            if name != "natural_log_exp_and_others":
                funcs.discard(AFT.Exp)
                funcs.discard(AFT.Ln)
        return tables

    bacc.get_activation_tables = _gat_one_set
    bacc._act_tbl_patched = True
import concourse.mybir as mybir
from concourse import tile
from concourse.masks import make_identity
from concourse.bass_utils import run_bass_kernel_spmd

F32 = mybir.dt.float32
BF16 = mybir.dt.bfloat16
AF = mybir.ActivationFunctionType
ALU = mybir.AluOpType
BF = ml_dtypes.bfloat16

# problem constants (hardcoded per harness rules)
B, P, N, D = 2, 8, 1024, 512
_last_res = None
HEADS, HD = 8, 64
INNER = HEADS * HD            # 512
E = 3 * INNER                 # 1536
EPS = 1e-5
SCALE = HD ** -0.5            # 0.125
NT = N // 128                 # 8 i-tiles
DC = D // 128                 # 4 d-chunks
G_PER_CORE = 2
N_CORES = 8

# host-side w1 column permutation: [q_c0, k_c0, v, q_c123, k_c567]
# qkT slot c (0-3 = q chunks, 4-7 = k chunks) -> column offset in w1p
QK_COL = {0: 0, 4: 128, 1: 768, 2: 896, 3: 1024, 5: 1152, 6: 1280, 7: 1408}
V_COL = 256  # v block at [256:768]


def build_graph(use_b1: bool):
    nc = bacc.Bacc("TRN2", target_bir_lowering=False, debug=False)

    x = nc.declare_dram_parameter("x", [G_PER_CORE, N, D], F32, isOutput=False)
    w1 = nc.declare_dram_parameter("w1", [D, E], BF16, isOutput=False)
    w2 = nc.declare_dram_parameter("w2", [INNER, D], BF16, isOutput=False)
    if use_b1:
        # b1p[:, j] = bias for permuted chunk j (12 chunks of 128)
        b1 = nc.declare_dram_parameter("b1", [128, E // 128], F32, isOutput=False)
        b1v = nc.declare_dram_parameter("b1v", [1, INNER], BF16, isOutput=False)
    out = nc.declare_dram_parameter("out", [G_PER_CORE, N, D], F32, isOutput=True)

    with tile.TileContext(nc) as tc, ExitStack() as ctx:
        const = ctx.enter_context(tc.tile_pool(name="const", bufs=1))
        ln_pool = ctx.enter_context(tc.tile_pool(name="ln", bufs=5))
        xt_pool = ctx.enter_context(tc.tile_pool(name="xt", bufs=2))
        qkv_pool = ctx.enter_context(tc.tile_pool(name="qkv", bufs=2))
        pt_pool = ctx.enter_context(tc.tile_pool(name="pt", bufs=10))
        ao_pool = ctx.enter_context(tc.tile_pool(name="ao", bufs=2))
        aux_pool = ctx.enter_context(tc.tile_pool(name="aux", bufs=3))
        out_pool = ctx.enter_context(tc.tile_pool(name="outp", bufs=4))
        # PSUM (8 banks): st 2x2 + ao 1x2 + mm 2x1 = 8
        ps_st = ctx.enter_context(tc.tile_pool(name="ps_st", bufs=2, space="PSUM"))
        ps_ao = ctx.enter_context(tc.tile_pool(name="ps_ao", bufs=1, space="PSUM"))
        ps_mm = ctx.enter_context(tc.tile_pool(name="ps_mm", bufs=2, space="PSUM"))

        # ---- PE warmup (HAM) : cheap small-N matmuls on a memset tile ----
        warm_src = const.tile([128, 64], BF16)
        nc.vector.memset(warm_src, 1.0)
        warm_f32 = const.tile([128, 64], F32)
        nc.vector.memset(warm_f32, 1.0)
        for i in range(16):
            wacc = ps_mm.tile([64, 64], F32, tag="mm", name=f"warm{i}")
            nc.tensor.matmul(wacc, warm_src, warm_src, start=True, stop=True)

        # ---- group-0 x prefetch ----
        x_tiles = {}
        for t in range(NT):
            x_t0 = ln_pool.tile([128, D], F32, tag=f"x_t{t}", bufs=1,
                                name=f"x_g0_t{t}")
            x_tiles[(0, t)] = x_t0
            nc.sync.dma_start(out=x_t0, in_=x[0, 128 * t:128 * (t + 1), :])

        # ---- weights: priority pieces ----
        w1_sb = const.tile([128, DC, E], BF16)
        w1r = w1.rearrange("(dc p) e -> p dc e", p=128)
        # piece A: q_c0 + k_c0 (cols 0:256) -> heads 0/1 can start
        nc.sync.dma_start(out=w1_sb[:, :, 0:256], in_=w1r[:, :, 0:256])
        # piece B: v block (cols 256:768)
        nc.sync.dma_start(out=w1_sb[:, :, 256:768], in_=w1r[:, :, 256:768])
        # piece C: remaining q/k chunks
        nc.sync.dma_start(out=w1_sb[:, :, 768:E], in_=w1r[:, :, 768:E])
        w2_sb = const.tile([128, DC, D], BF16)
        nc.sync.dma_start(out=w2_sb, in_=w2.rearrange("(kc p) d -> p kc d", p=128))
        eps_sb = const.tile([128, 1], F32)
        nc.vector.memset(eps_sb, EPS)
        ones_row = const.tile([1, 128], BF16)
        nc.vector.memset(ones_row, 1.0)
        ident = const.tile([128, 128], BF16)
        make_identity(nc, ident)
        if use_b1:
            b1_sb = const.tile([128, E // 128], F32)
            nc.sync.dma_start(out=b1_sb, in_=b1[:, :])
            b1v_sb = const.tile([1, INNER], BF16)
            nc.sync.dma_start(out=b1v_sb, in_=b1v[:, :])

        state = {}

        def phase_ln(g):
            # LayerNorm; rstd computed in two half-batches so the first
            # transposes start ~3us earlier.  g0 transposes on PE (lead-in,
            # PE otherwise idle); g1 transposes on the DMA xbar (runs under
            # att(0) where PE is the bottleneck and DMA is ~idle).
            xhatT_lo = xt_pool.tile([128, DC, N // 2], BF16, tag="xhatT_lo",
                                    name=f"xhatT_lo_g{g}")
            xhatT_hi = xt_pool.tile([128, DC, N // 2], BF16, tag="xhatT_hi",
                                    name=f"xhatT_hi_g{g}")
            xhatT = (xhatT_lo, xhatT_hi)
            x_ts = []
            mv_all = ln_pool.tile([128, NT, 2], F32, tag="mv_all",
                                  name=f"mv_all_g{g}")
            rstds = {}
            for half in range(2):
                for t in range(4 * half, 4 * half + 4):
                    if (g, t) in x_tiles:
                        x_t = x_tiles[(g, t)]
                    else:
                        x_t = ln_pool.tile([128, D], F32, tag=f"x_t{t}",
                                           bufs=1, name=f"x_g{g}_t{t}")
                        nc.sync.dma_start(out=x_t,
                                          in_=x[g, 128 * t:128 * (t + 1), :])
                    x_ts.append(x_t)
                    stats = ln_pool.tile([128, 6], F32, tag="stats")
                    nc.vector.bn_stats(out=stats, in_=x_t)
                    nc.vector.bn_aggr(out=mv_all[:, t, :], in_=stats)
                    if g == 0:
                        # PE heartbeat: keeps HAM warm through the DVE-serial
                        # stats phase so transposes/first matmuls run at 2.4GHz
                        hb = ps_mm.tile([64, 2], F32, tag="mm",
                                        name=f"hb_g{g}_t{t}")
                        nc.tensor.matmul(hb, warm_f32[:, 0:64],
                                         mv_all[:, t, :], start=True, stop=True)
                lnv = ln_pool.tile([128, 4], F32, tag=f"lnv{half}",
                                   name=f"lnv_g{g}_{half}")
                sl = mv_all.rearrange("p t s -> p (t s)")[:, 8 * half + 1:8 * half + 8:2]
                nc.scalar.activation(out=lnv, in_=sl,
                                     func=AF.Ln, bias=eps_sb, scale=1.0)
                rstd = ln_pool.tile([128, 4], F32, tag=f"rstd{half}",
                                    name=f"rstd_g{g}_{half}")
                nc.scalar.activation(out=rstd, in_=lnv, func=AF.Exp, scale=-0.5)
                rstds[half] = rstd
            for t in range(NT):
                xhat = ln_pool.tile([128, D], BF16, tag="xhat")
                nc.vector.tensor_scalar(out=xhat, in0=x_ts[t],
                                        scalar1=mv_all[:, t, 0:1],
                                        scalar2=rstds[t // 4][:, t % 4:t % 4 + 1],
                                        op0=ALU.subtract, op1=ALU.mult)
                tp = ps_mm.tile([128, 512], BF16, tag="mm",
                                name=f"tp_g{g}_t{t}")
                for dc in range(DC):
                    nc.tensor.transpose(tp[:, 128 * dc:128 * (dc + 1)],
                                        xhat[:, 128 * dc:128 * (dc + 1)],
                                        ident)
                half, tt = t // 4, t % 4
                dst = xhatT[half][:, :, 128 * tt:128 * (tt + 1)]
                if g == 0:
                    # pre-stream: ACT is idle and these copies are strictly
                    # upstream of the first exp, so the ACT FIFO stays safe;
                    # frees ~3.3us of the critical DVE lead-in chain
                    nc.scalar.activation(
                        out=dst, in_=tp.rearrange("p (b c) -> p b c", b=DC),
                        func=AF.Copy)
                else:
                    nc.vector.tensor_copy(
                        out=dst, in_=tp.rearrange("p (b c) -> p b c", b=DC))
            state[g] = dict(xhatT=xhatT)

        def qk_chunks(g, chunks):
            # qkT slots: 0-3 q chunks, 4-7 k chunks; transposed layout [e,i]
            xhatT = state[g]["xhatT"]
            if "qkT" not in state[g]:
                qkT = qkv_pool.tile([128, 8, N], BF16, tag="qkT",
                                    name=f"qkT_g{g}")
                state[g]["qkT"] = qkT
            qkT = state[g]["qkT"]
            for c in chunks:
                col = QK_COL[c]
                accs = [ps_mm.tile([128, 512], F32, tag="mm",
                                   name=f"qk_g{g}_c{c}_{ic}") for ic in range(2)]
                for dc in range(DC):
                    for ic in range(2):
                        nc.tensor.matmul(
                            accs[ic], w1_sb[:, dc, col:col + 128],
                            xhatT[ic][:, dc, :],
                            start=(dc == 0), stop=(dc == DC - 1))
                for ic in range(2):
                    dst = qkT[:, c, 512 * ic:512 * (ic + 1)]
                    if use_b1:
                        cidx = col // 128
                        nc.vector.tensor_scalar(out=dst, in0=accs[ic],
                                                scalar1=b1_sb[:, cidx:cidx + 1],
                                                scalar2=None, op0=ALU.add)
                    else:
                        nc.vector.tensor_copy(out=dst, in_=accs[ic])

        def v_alloc(g):
            if "v_sb" not in state[g]:
                v_sb = qkv_pool.tile([128, NT, HEADS, HD + 1], BF16,
                                     tag="v_sb", name=f"v_sb_g{g}")
                state[g]["v_sb"] = v_sb
            return state[g]["v_sb"]

        def v_proj(g):
            xhatT = state[g]["xhatT"]
            # V natural layout + ones column per head (softmax denominator row)
            v_sb = v_alloc(g)
            nc.vector.memset(v_sb[:, :, :, HD:HD + 1], 1.0)
            for t in range(NT):
                acc = ps_mm.tile([128, 512], F32, tag="mm")
                half, tt = t // 4, t % 4
                for dc in range(DC):
                    nc.tensor.matmul(
                        acc, xhatT[half][:, dc, 128 * tt:128 * (tt + 1)],
                        w1_sb[:, dc, V_COL:V_COL + INNER],
                        start=(dc == 0), stop=(dc == DC - 1 and not use_b1))
                if use_b1:
                    nc.tensor.matmul(acc, ones_row, b1v_sb,
                                     start=False, stop=True)
                nc.vector.tensor_copy(
                    out=v_sb[:, t, :, 0:HD],
                    in_=acc.rearrange("p (h c) -> p h c", h=HEADS))

        def att_head_st(g, h):
            # ST + exp for all jt of head h (no V dependency) -> pt tiles
            qkT = state[g]["qkT"]
            hp, rlo = h // 2, 64 * (h % 2)
            pts = []
            for jt in range(NT):
                st = ps_st.tile([128, N], F32, tag="st",
                                name=f"st_g{g}_h{h}_j{jt}")
                for ic in range(2):
                    nc.tensor.matmul(
                        st[:, 512 * ic:512 * (ic + 1)],
                        qkT[rlo:rlo + 64, 4 + hp, 128 * jt:128 * (jt + 1)],
                        qkT[rlo:rlo + 64, hp, 512 * ic:512 * (ic + 1)],
                        start=True, stop=True, tile_position=(rlo, 0))
                pt = pt_pool.tile([128, N], BF16, tag="pt",
                                  name=f"pt_g{g}_h{h}_j{jt}")
                nc.scalar.activation(out=pt, in_=st, func=AF.Exp, scale=SCALE)
                for ic in range(2):
                    nc.tensor.matmul(
                        state[g][f"aot_h{h}"][0:65, 512 * ic:512 * (ic + 1)],
                        v_alloc(g)[:, jt, h, :],
                        pt[:, 512 * ic:512 * (ic + 1)],
                        start=(jt == 0), stop=(jt == NT - 1))

        def att_head(g, h):
            qkT, v_sb = state[g]["qkT"], v_alloc(g)
            if "aoT" not in state[g]:
                state[g]["aoT"] = []
                for kc in range(DC):
                    aoT_kc = ao_pool.tile([128, N], BF16, tag=f"aoT{kc}",
                                          name=f"aoT_g{g}_kc{kc}")
                    state[g]["aoT"].append(aoT_kc)
            aoT = state[g]["aoT"]
            hp, rlo = h // 2, 64 * (h % 2)
            aot = ps_ao.tile([128, N], F32, tag="ao", name=f"aot_g{g}_h{h}")
            for jt in range(NT):
                st = ps_st.tile([128, N], F32, tag="st",
                                name=f"st_g{g}_h{h}_j{jt}")
                for ic in range(2):
                    nc.tensor.matmul(
                        st[:, 512 * ic:512 * (ic + 1)],
                        qkT[rlo:rlo + 64, 4 + hp, 128 * jt:128 * (jt + 1)],
                        qkT[rlo:rlo + 64, hp, 512 * ic:512 * (ic + 1)],
                        start=True, stop=True, tile_position=(rlo, 0))
                pt = pt_pool.tile([128, N], BF16, tag="pt",
                                  name=f"pt_g{g}_h{h}_j{jt}")
                nc.scalar.activation(out=pt, in_=st, func=AF.Exp, scale=SCALE)
                for ic in range(2):
                    # M=65: V cols 0-63 + ones col -> row 64 accumulates the
                    # softmax denominator.
                    nc.tensor.matmul(
                        aot[0:65, 512 * ic:512 * (ic + 1)],
                        v_sb[:, jt, h, :],
                        pt[:, 512 * ic:512 * (ic + 1)],
                        start=(jt == 0), stop=(jt == NT - 1))
            # per-head normalization, entirely off the ACT engine.  Order:
            # denominator row (f32, PSUM) -> recip -> bf16 -> gpsimd
            # broadcast, with the raw copy overlapping the broadcast; the
            # PSUM ao bank frees after the raw copy.
            raw = aux_pool.tile([65, N], BF16, tag="raw",
                                name=f"raw_g{g}_h{h}")
            nc.vector.tensor_copy(out=raw, in_=aot[0:65, :])
            lrow = aux_pool.tile([1, N], F32, tag="lrow",
                                 name=f"lrow_g{g}_h{h}")
            nc.vector.tensor_copy(out=lrow, in_=raw[64:65, :])
            recip = aux_pool.tile([1, N], F32, tag="recip",
                                  name=f"recip_g{g}_h{h}")
            nc.vector.reciprocal_approx_fast(out=recip, in_=lrow)
            recip_bf = aux_pool.tile([1, N], BF16, tag="recipbf",
                                     name=f"recipbf_g{g}_h{h}")
            nc.vector.tensor_copy(out=recip_bf, in_=recip)
            bc = aux_pool.tile([64, N], BF16, tag="bc", name=f"bc_g{g}_h{h}")
            nc.gpsimd.partition_broadcast(out_ap=bc, in_ap=recip_bf)
            nc.vector.tensor_tensor(out=aoT[hp][rlo:rlo + 64, :],
                                    in0=raw[0:64, :], in1=bc, op=ALU.mult)

        def fin_pass_a(g):
            # kc0-2 accumulate in PSUM (ready after h5-norm), copy to SBUF
            aoT = state[g]["aoT"]
            o_accs = []
            state[g]["o_acc"] = o_accs
            for t in range(NT):
                acc = ps_mm.tile([128, 512], F32, tag="mm")
                for kc in range(3):
                    nc.tensor.matmul(acc, aoT[kc][:, 128 * t:128 * (t + 1)],
                                     w2_sb[:, kc, :],
                                     start=(kc == 0), stop=(kc == 2))
                o_acc = out_pool.tile([128, D], F32, tag=f"o_acc{t}",
                                      bufs=1, name=f"o_acc_g{g}_t{t}")
                o_accs.append(o_acc)
                nc.vector.tensor_copy(out=o_acc, in_=acc)

        def fin_pass_b(g):
            # kc3 (heads 6/7) + DVE add + output DMA: the only trailing part
            aoT = state[g]["aoT"]
            o_accs = state[g]["o_acc"]
            for t in range(NT):
                acc = ps_mm.tile([128, 512], F32, tag="mm")
                nc.tensor.matmul(acc, aoT[3][:, 128 * t:128 * (t + 1)],
                                 w2_sb[:, 3, :], start=True, stop=True)
                o_t = out_pool.tile([128, D], F32, tag="o_t", bufs=2)
                nc.vector.tensor_tensor(out=o_t, in0=acc, in1=o_accs[t],
                                        op=ALU.add)
                nc.sync.dma_start(out=out[g, 128 * t:128 * (t + 1), :],
                                  in_=o_t)

        # ---------------- emission order = scheduler priority ----------------
        phase_ln(0)
        qk_chunks(0, [0, 4])
        v_proj(0)
        att_head(0, 0)
        att_head(0, 1)
        qk_chunks(0, [1, 5])
        att_head(0, 2)
        att_head(0, 3)
        qk_chunks(0, [2, 6])
        att_head(0, 4)
        # prefetch x for group 1 mid-stream
        for t in range(NT):
            x_t1 = ln_pool.tile([128, D], F32, tag=f"x_t{t}", bufs=1,
                                name=f"x_g1_t{t}")
            x_tiles[(1, t)] = x_t1
            nc.sync.dma_start(out=x_t1, in_=x[1, 128 * t:128 * (t + 1), :])
        att_head(0, 5)
        qk_chunks(0, [3, 7])
        att_head(0, 6)
        att_head(0, 7)
        phase_ln(1)
        qk_chunks(1, [0, 4])
        v_proj(1)
        qk_chunks(1, [1, 5])
        qk_chunks(1, [2, 6])
        qk_chunks(1, [3, 7])
        att_head(1, 0)
        fin_pass_a(0)
        att_head(1, 1)
        att_head(1, 2)
        att_head(1, 3)
        fin_pass_b(0)
        att_head(1, 4)
        att_head(1, 5)
        att_head(1, 6)
        fin_pass_a(1)
        att_head(1, 7)
        # warm-keeper: PE heartbeat through the last norm chain so the
        # trailing kc3 matmuls run at 2.4GHz
        for i in range(40):
            wk = ps_mm.tile([64, 64], F32, tag="mm", name=f"wk{i}")
            nc.tensor.matmul(wk, warm_src[:, 0:64], warm_src[:, 0:64],
                             start=True, stop=True, skip_group_check=True)
        fin_pass_b(1)

    nc.compile()
    return nc


def kernel(x, ln_w, ln_b, w_qkv, w_out):
    x = np.asarray(x, dtype=np.float32)
    ln_w = np.asarray(ln_w, dtype=np.float32)
    ln_b = np.asarray(ln_b, dtype=np.float32)
    w_qkv = np.asarray(w_qkv, dtype=np.float32)
    w_out = np.asarray(w_out, dtype=np.float32)

    # host-side weight folding (LN affine into QKV weights)
    w1 = (w_qkv * ln_w[None, :]).T.astype(BF)            # [D, E]
    b1 = (w_qkv @ ln_b).astype(np.float32)               # [E]
    w2 = w_out.T.astype(BF)                              # [INNER, D]
    use_b1 = bool(np.any(b1))

    # column permutation: [q_c0, k_c0, v, q_c123, k_c567]
    perm = np.concatenate([
        np.arange(0, 128),                  # q chunk 0
        np.arange(512, 640),                # k chunk 0
        np.arange(1024, 1536),              # v
        np.arange(128, 512),                # q chunks 1-3
        np.arange(640, 1024),               # k chunks 5-7
    ])
    w1p = np.ascontiguousarray(w1[:, perm])
    b1p = b1[perm]

    nc = build_graph(use_b1)

    xg = x.reshape(B * P, N, D)
    in_maps = []
    for core in range(N_CORES):
        m = {
            "x": np.ascontiguousarray(xg[G_PER_CORE * core:G_PER_CORE * (core + 1)]),
            "w1": w1p,
            "w2": w2,
        }
        if use_b1:
            m["b1"] = b1p.reshape(E // 128, 128).T.astype(np.float32).copy()
            m["b1v"] = b1p[V_COL:V_COL + INNER].reshape(1, INNER).astype(BF)
        in_maps.append(m)

    trace = bool(int(os.environ.get("KERNEL_TRACE", "0")))
    if trace:
        try:
            import ntff_shim
            ntff_shim.install()
        except Exception as e:
            print(f"ntff shim unavailable: {e}")
            trace = False
    res = run_bass_kernel_spmd(nc, in_maps, list(range(N_CORES)), trace=trace,
                               tmpdir=os.environ.get("KERNEL_TRACE_DIR"))
    global _last_res
    _last_res = res
    if res.exec_time_ns is not None:
        print(f"HW exec time: {res.exec_time_ns} ns")
    out = np.concatenate([r["out"] for r in res.results], axis=0)
    return out.reshape(B, P, N, D)


# revision 12
# speedup vs baseline: 1.0108x; 1.0108x over previous
"""Trainium2 Bass kernel for nn_Attention (LN + QKV + 8-head attention + out-proj).

Sharding: data-parallel over the 16 (b,p) groups -> 2 groups per core, weights
replicated, no collectives.

Per-core dataflow (matmul compute bf16, f32 accumulation):
  LN per i-tile (bn_stats/bn_aggr, batched ln/exp rstd) -> xhat bf16
  PE-transpose -> xhatT [d,i]
  QKV: qkT [e,i] chunks (w1 stationary); V [j,c] natural (+ ones column so
       the PV matmul accumulates the softmax denominator in row 64)
  Attention per head: ST = kT.T @ qT (K=64, row-paired tile_position);
       pt = exp(ST/8) on ACT (the pacing stream, 128 x ~1.1us);
       aot[0:65] = V.T @ pt accumulated over j-chunks in PSUM
  Per-head normalization entirely off the ACT engine: raw copy to SBUF
       (frees the PSUM bank early) -> DVE reciprocal_approx_fast on the
       denominator row -> gpsimd partition_broadcast -> fused DVE multiply
  final [i,d] = aoT-chunks.T @ w2, DMA out.

Scheduling: w1 columns host-permuted to [q0|k0|v|rest] so heads 0/1 start
~8us after the x DMA; group-1 x prefetched mid-stream; emission order
interleaves qkT chunk-pairs with the attention heads that need them.
"""
import sys
import os

sys.path.insert(0, "/opt/trn_rl_repo")

import numpy as np
import ml_dtypes
from contextlib import ExitStack

import concourse.bass as bass
import concourse.bacc as bacc

# Steer Bacc's activation-table selection to the set containing BOTH exp and
# ln ("natural_log_exp_and_others") so the kernel runs with zero mid-stream
# ACT_TABLE_LOAD swaps.
if not getattr(bacc, "_act_tbl_patched", False):
    _orig_gat = bacc.get_activation_tables

    def _gat_one_set(arch):
        tables = {k: set(v) for k, v in _orig_gat(arch).items()}
        AFT = mybir.ActivationFunctionType
        for name, funcs in tables.items():
            if name != "natural_log_exp_and_others":
                funcs.discard(AFT.Exp)
                funcs.discard(AFT.Ln)
        return tables

    bacc.get_activation_tables = _gat_one_set
    bacc._act_tbl_patched = True
import concourse.mybir as mybir
from concourse import tile
from concourse.masks import make_identity
from concourse.bass_utils import run_bass_kernel_spmd

F32 = mybir.dt.float32
BF16 = mybir.dt.bfloat16
AF = mybir.ActivationFunctionType
ALU = mybir.AluOpType
BF = ml_dtypes.bfloat16

# problem constants (hardcoded per harness rules)
B, P, N, D = 2, 8, 1024, 512
_last_res = None
HEADS, HD = 8, 64
INNER = HEADS * HD            # 512
E = 3 * INNER                 # 1536
EPS = 1e-5
SCALE = HD ** -0.5            # 0.125
NT = N // 128                 # 8 i-tiles
DC = D // 128                 # 4 d-chunks
G_PER_CORE = 2
N_CORES = 8

# host-side w1 column permutation: [q_c0, k_c0, v, q_c123, k_c567]
# qkT slot c (0-3 = q chunks, 4-7 = k chunks) -> column offset in w1p
QK_COL = {0: 0, 4: 128, 1: 768, 2: 896, 3: 1024, 5: 1152, 6: 1280, 7: 1408}
V_COL = 256  # v block at [256:768]


def build_graph(use_b1: bool):
    nc = bacc.Bacc("TRN2", target_bir_lowering=False, debug=False)

    x = nc.declare_dram_parameter("x", [G_PER_CORE, N, D], F32, isOutput=False)
    w1 = nc.declare_dram_parameter("w1", [D, E], BF16, isOutput=False)
    w2 = nc.declare_dram_parameter("w2", [INNER, D], BF16, isOutput=False)
    if use_b1:
        # b1p[:, j] = bias for permuted chunk j (12 chunks of 128)
        b1 = nc.declare_dram_parameter("b1", [128, E // 128], F32, isOutput=False)
        b1v = nc.declare_dram_parameter("b1v", [1, INNER], BF16, isOutput=False)
    out = nc.declare_dram_parameter("out", [G_PER_CORE, N, D], F32, isOutput=True)

    with tile.TileContext(nc) as tc, ExitStack() as ctx:
        const = ctx.enter_context(tc.tile_pool(name="const", bufs=1))
        ln_pool = ctx.enter_context(tc.tile_pool(name="ln", bufs=5))
        xt_pool = ctx.enter_context(tc.tile_pool(name="xt", bufs=2))
        qkv_pool = ctx.enter_context(tc.tile_pool(name="qkv", bufs=2))
        pt_pool = ctx.enter_context(tc.tile_pool(name="pt", bufs=10))
        ao_pool = ctx.enter_context(tc.tile_pool(name="ao", bufs=2))
        aux_pool = ctx.enter_context(tc.tile_pool(name="aux", bufs=3))
        out_pool = ctx.enter_context(tc.tile_pool(name="outp", bufs=4))
        # PSUM (8 banks): st 2x2 + ao 1x2 + mm 2x1 = 8
        ps_st = ctx.enter_context(tc.tile_pool(name="ps_st", bufs=2, space="PSUM"))
        ps_ao = ctx.enter_context(tc.tile_pool(name="ps_ao", bufs=1, space="PSUM"))
        ps_mm = ctx.enter_context(tc.tile_pool(name="ps_mm", bufs=2, space="PSUM"))

        # ---- PE warmup (HAM) : cheap small-N matmuls on a memset tile ----
        warm_src = const.tile([128, 64], BF16)
        nc.vector.memset(warm_src, 1.0)
        warm_f32 = const.tile([128, 64], F32)
        nc.vector.memset(warm_f32, 1.0)
        for i in range(16):
            wacc = ps_mm.tile([64, 64], F32, tag="mm", name=f"warm{i}")
            nc.tensor.matmul(wacc, warm_src, warm_src, start=True, stop=True)

        # ---- group-0 x prefetch ----
        x_tiles = {}
        for t in range(NT):
            x_t0 = ln_pool.tile([128, D], F32, tag=f"x_t{t}", bufs=1,
                                name=f"x_g0_t{t}")
            x_tiles[(0, t)] = x_t0
            nc.sync.dma_start(out=x_t0, in_=x[0, 128 * t:128 * (t + 1), :])

        # ---- weights: priority pieces ----
        w1_sb = const.tile([128, DC, E], BF16)
        w1r = w1.rearrange("(dc p) e -> p dc e", p=128)
        # piece A: q_c0 + k_c0 (cols 0:256) -> heads 0/1 can start
        nc.sync.dma_start(out=w1_sb[:, :, 0:256], in_=w1r[:, :, 0:256])
        # piece B: v block (cols 256:768)
        nc.sync.dma_start(out=w1_sb[:, :, 256:768], in_=w1r[:, :, 256:768])
        # piece C: remaining q/k chunks
        nc.sync.dma_start(out=w1_sb[:, :, 768:E], in_=w1r[:, :, 768:E])
        w2_sb = const.tile([128, DC, D], BF16)
        nc.sync.dma_start(out=w2_sb, in_=w2.rearrange("(kc p) d -> p kc d", p=128))
        eps_sb = const.tile([128, 1], F32)
        nc.vector.memset(eps_sb, EPS)
        ones_row = const.tile([1, 128], BF16)
        nc.vector.memset(ones_row, 1.0)
        ident = const.tile([128, 128], BF16)
        make_identity(nc, ident)
        if use_b1:
            b1_sb = const.tile([128, E // 128], F32)
            nc.sync.dma_start(out=b1_sb, in_=b1[:, :])
            b1v_sb = const.tile([1, INNER], BF16)
            nc.sync.dma_start(out=b1v_sb, in_=b1v[:, :])

        state = {}

        def phase_ln(g):
            # LayerNorm; rstd computed in two half-batches so the first
            # transposes start ~3us earlier.  g0 transposes on PE (lead-in,
            # PE otherwise idle); g1 transposes on the DMA xbar (runs under
            # att(0) where PE is the bottleneck and DMA is ~idle).
            xhatT_lo = xt_pool.tile([128, DC, N // 2], BF16, tag="xhatT_lo",
                                    name=f"xhatT_lo_g{g}")
            xhatT_hi = xt_pool.tile([128, DC, N // 2], BF16, tag="xhatT_hi",
                                    name=f"xhatT_hi_g{g}")
            xhatT = (xhatT_lo, xhatT_hi)
            x_ts = []
            mv_all = ln_pool.tile([128, NT, 2], F32, tag="mv_all",
                                  name=f"mv_all_g{g}")
            rstds = {}
            for half in range(2):
                for t in range(4 * half, 4 * half + 4):
                    if (g, t) in x_tiles:
                        x_t = x_tiles[(g, t)]
                    else:
                        x_t = ln_pool.tile([128, D], F32, tag=f"x_t{t}",
                                           bufs=1, name=f"x_g{g}_t{t}")
                        nc.sync.dma_start(out=x_t,
                                          in_=x[g, 128 * t:128 * (t + 1), :])
                    x_ts.append(x_t)
                    stats = ln_pool.tile([128, 6], F32, tag="stats")
                    nc.vector.bn_stats(out=stats, in_=x_t)
                    nc.vector.bn_aggr(out=mv_all[:, t, :], in_=stats)
                    if g == 0:
                        # PE heartbeat: keeps HAM warm through the DVE-serial
                        # stats phase so transposes/first matmuls run at 2.4GHz
                        hb = ps_mm.tile([64, 2], F32, tag="mm",
                                        name=f"hb_g{g}_t{t}")
                        nc.tensor.matmul(hb, warm_f32[:, 0:64],
                                         mv_all[:, t, :], start=True, stop=True)
                lnv = ln_pool.tile([128, 4], F32, tag=f"lnv{half}",
                                   name=f"lnv_g{g}_{half}")
                sl = mv_all.rearrange("p t s -> p (t s)")[:, 8 * half + 1:8 * half + 8:2]
                nc.scalar.activation(out=lnv, in_=sl,
                                     func=AF.Ln, bias=eps_sb, scale=1.0)
                rstd = ln_pool.tile([128, 4], F32, tag=f"rstd{half}",
                                    name=f"rstd_g{g}_{half}")
                nc.scalar.activation(out=rstd, in_=lnv, func=AF.Exp, scale=-0.5)
                rstds[half] = rstd
            for t in range(NT):
                xhat = ln_pool.tile([128, D], BF16, tag="xhat")
                nc.vector.tensor_scalar(out=xhat, in0=x_ts[t],
                                        scalar1=mv_all[:, t, 0:1],
                                        scalar2=rstds[t // 4][:, t % 4:t % 4 + 1],
                                        op0=ALU.subtract, op1=ALU.mult)
                tp = ps_mm.tile([128, 512], BF16, tag="mm",
                                name=f"tp_g{g}_t{t}")
                for dc in range(DC):
                    nc.tensor.transpose(tp[:, 128 * dc:128 * (dc + 1)],
                                        xhat[:, 128 * dc:128 * (dc + 1)],
                                        ident)
                half, tt = t // 4, t % 4
                nc.vector.tensor_copy(
                    out=xhatT[half][:, :, 128 * tt:128 * (tt + 1)],
                    in_=tp.rearrange("p (b c) -> p b c", b=DC))
            state[g] = dict(xhatT=xhatT)

        def qk_chunks(g, chunks):
            # qkT slots: 0-3 q chunks, 4-7 k chunks; transposed layout [e,i]
            xhatT = state[g]["xhatT"]
            if "qkT" not in state[g]:
                qkT = qkv_pool.tile([128, 8, N], BF16, tag="qkT",
                                    name=f"qkT_g{g}")
                state[g]["qkT"] = qkT
            qkT = state[g]["qkT"]
            for c in chunks:
                col = QK_COL[c]
                accs = [ps_mm.tile([128, 512], F32, tag="mm",
                                   name=f"qk_g{g}_c{c}_{ic}") for ic in range(2)]
                for dc in range(DC):
                    for ic in range(2):
                        nc.tensor.matmul(
                            accs[ic], w1_sb[:, dc, col:col + 128],
                            xhatT[ic][:, dc, :],
                            start=(dc == 0), stop=(dc == DC - 1))
                for ic in range(2):
                    dst = qkT[:, c, 512 * ic:512 * (ic + 1)]
                    if use_b1:
                        cidx = col // 128
                        nc.vector.tensor_scalar(out=dst, in0=accs[ic],
                                                scalar1=b1_sb[:, cidx:cidx + 1],
                                                scalar2=None, op0=ALU.add)
                    else:
                        nc.vector.tensor_copy(out=dst, in_=accs[ic])

        def v_alloc(g):
            if "v_sb" not in state[g]:
                v_sb = qkv_pool.tile([128, NT, HEADS, HD + 1], BF16,
                                     tag="v_sb", name=f"v_sb_g{g}")
                state[g]["v_sb"] = v_sb
            return state[g]["v_sb"]

        def v_proj(g):
            xhatT = state[g]["xhatT"]
            # V natural layout + ones column per head (softmax denominator row)
            v_sb = v_alloc(g)
            nc.vector.memset(v_sb[:, :, :, HD:HD + 1], 1.0)
            for t in range(NT):
                acc = ps_mm.tile([128, 512], F32, tag="mm")
                half, tt = t // 4, t % 4
                for dc in range(DC):
                    nc.tensor.matmul(
                        acc, xhatT[half][:, dc, 128 * tt:128 * (tt + 1)],
                        w1_sb[:, dc, V_COL:V_COL + INNER],
                        start=(dc == 0), stop=(dc == DC - 1 and not use_b1))
                if use_b1:
                    nc.tensor.matmul(acc, ones_row, b1v_sb,
                                     start=False, stop=True)
                nc.vector.tensor_copy(
                    out=v_sb[:, t, :, 0:HD],
                    in_=acc.rearrange("p (h c) -> p h c", h=HEADS))

        def att_head_st(g, h):
            # ST + exp for all jt of head h (no V dependency) -> pt tiles
            qkT = state[g]["qkT"]
            hp, rlo = h // 2, 64 * (h % 2)
            pts = []
            for jt in range(NT):
                st = ps_st.tile([128, N], F32, tag="st",
                                name=f"st_g{g}_h{h}_j{jt}")
                for ic in range(2):
                    nc.tensor.matmul(
                        st[:, 512 * ic:512 * (ic + 1)],
                        qkT[rlo:rlo + 64, 4 + hp, 128 * jt:128 * (jt + 1)],
                        qkT[rlo:rlo + 64, hp, 512 * ic:512 * (ic + 1)],
                        start=True, stop=True, tile_position=(rlo, 0))
                pt = pt_pool.tile([128, N], BF16, tag="pt",
                                  name=f"pt_g{g}_h{h}_j{jt}")
                nc.scalar.activation(out=pt, in_=st, func=AF.Exp, scale=SCALE)
                for ic in range(2):
                    nc.tensor.matmul(
                        state[g][f"aot_h{h}"][0:65, 512 * ic:512 * (ic + 1)],
                        v_alloc(g)[:, jt, h, :],
                        pt[:, 512 * ic:512 * (ic + 1)],
                        start=(jt == 0), stop=(jt == NT - 1))

        def att_head(g, h):
            qkT, v_sb = state[g]["qkT"], v_alloc(g)
            if "aoT" not in state[g]:
                state[g]["aoT"] = []
                for kc in range(DC):
                    aoT_kc = ao_pool.tile([128, N], BF16, tag=f"aoT{kc}",
                                          name=f"aoT_g{g}_kc{kc}")
                    state[g]["aoT"].append(aoT_kc)
            aoT = state[g]["aoT"]
            hp, rlo = h // 2, 64 * (h % 2)
            aot = ps_ao.tile([128, N], F32, tag="ao", name=f"aot_g{g}_h{h}")
            for jt in range(NT):
                st = ps_st.tile([128, N], F32, tag="st",
                                name=f"st_g{g}_h{h}_j{jt}")
                for ic in range(2):
                    nc.tensor.matmul(
                        st[:, 512 * ic:512 * (ic + 1)],
                        qkT[rlo:rlo + 64, 4 + hp, 128 * jt:128 * (jt + 1)],
                        qkT[rlo:rlo + 64, hp, 512 * ic:512 * (ic + 1)],
                        start=True, stop=True, tile_position=(rlo, 0))
                pt = pt_pool.tile([128, N], BF16, tag="pt",
                                  name=f"pt_g{g}_h{h}_j{jt}")
                nc.scalar.activation(out=pt, in_=st, func=AF.Exp, scale=SCALE)
                for ic in range(2):
                    # M=65: V cols 0-63 + ones col -> row 64 accumulates the
                    # softmax denominator.
                    nc.tensor.matmul(
                        aot[0:65, 512 * ic:512 * (ic + 1)],
                        v_sb[:, jt, h, :],
                        pt[:, 512 * ic:512 * (ic + 1)],
                        start=(jt == 0), stop=(jt == NT - 1))
            # per-head normalization, entirely off the ACT engine.  Order:
            # denominator row (f32, PSUM) -> recip -> bf16 -> gpsimd
            # broadcast, with the raw copy overlapping the broadcast; the
            # PSUM ao bank frees after the raw copy.
            raw = aux_pool.tile([65, N], BF16, tag="raw",
                                name=f"raw_g{g}_h{h}")
            nc.vector.tensor_copy(out=raw, in_=aot[0:65, :])
            lrow = aux_pool.tile([1, N], F32, tag="lrow",
                                 name=f"lrow_g{g}_h{h}")
            nc.vector.tensor_copy(out=lrow, in_=raw[64:65, :])
            recip = aux_pool.tile([1, N], F32, tag="recip",
                                  name=f"recip_g{g}_h{h}")
            nc.vector.reciprocal_approx_fast(out=recip, in_=lrow)
            recip_bf = aux_pool.tile([1, N], BF16, tag="recipbf",
                                     name=f"recipbf_g{g}_h{h}")
            nc.vector.tensor_copy(out=recip_bf, in_=recip)
            bc = aux_pool.tile([64, N], BF16, tag="bc", name=f"bc_g{g}_h{h}")
            nc.gpsimd.partition_broadcast(out_ap=bc, in_ap=recip_bf)
            nc.vector.tensor_tensor(out=aoT[hp][rlo:rlo + 64, :],
                                    in0=raw[0:64, :], in1=bc, op=ALU.mult)

        def fin_pass_a(g):
            # kc0-2 accumulate in PSUM (ready after h5-norm), copy to SBUF
            aoT = state[g]["aoT"]
            o_accs = []
            state[g]["o_acc"] = o_accs
            for t in range(NT):
                acc = ps_mm.tile([128, 512], F32, tag="mm")
                for kc in range(3):
                    nc.tensor.matmul(acc, aoT[kc][:, 128 * t:128 * (t + 1)],
                                     w2_sb[:, kc, :],
                                     start=(kc == 0), stop=(kc == 2))
                o_acc = out_pool.tile([128, D], F32, tag=f"o_acc{t}",
                                      bufs=1, name=f"o_acc_g{g}_t{t}")
                o_accs.append(o_acc)
                nc.vector.tensor_copy(out=o_acc, in_=acc)

        def fin_pass_b(g):
            # kc3 (heads 6/7) + DVE add + output DMA: the only trailing part
            aoT = state[g]["aoT"]
            o_accs = state[g]["o_acc"]
            for t in range(NT):
                acc = ps_mm.tile([128, 512], F32, tag="mm")
                nc.tensor.matmul(acc, aoT[3][:, 128 * t:128 * (t + 1)],
                                 w2_sb[:, 3, :], start=True, stop=True)
                o_t = out_pool.tile([128, D], F32, tag="o_t", bufs=2)
                nc.vector.tensor_tensor(out=o_t, in0=acc, in1=o_accs[t],
                                        op=ALU.add)
                nc.sync.dma_start(out=out[g, 128 * t:128 * (t + 1), :],
                                  in_=o_t)

        # ---------------- emission order = scheduler priority ----------------
        phase_ln(0)
        qk_chunks(0, [0, 4])
        v_proj(0)
        att_head(0, 0)
        att_head(0, 1)
        qk_chunks(0, [1, 5])
        att_head(0, 2)
        att_head(0, 3)
        qk_chunks(0, [2, 6])
        att_head(0, 4)
        # prefetch x for group 1 mid-stream
        for t in range(NT):
            x_t1 = ln_pool.tile([128, D], F32, tag=f"x_t{t}", bufs=1,
                                name=f"x_g1_t{t}")
            x_tiles[(1, t)] = x_t1
            nc.sync.dma_start(out=x_t1, in_=x[1, 128 * t:128 * (t + 1), :])
        att_head(0, 5)
        qk_chunks(0, [3, 7])
        att_head(0, 6)
        att_head(0, 7)
        phase_ln(1)
        qk_chunks(1, [0, 4])
        v_proj(1)
        qk_chunks(1, [1, 5])
        qk_chunks(1, [2, 6])
        qk_chunks(1, [3, 7])
        att_head(1, 0)
        fin_pass_a(0)
        att_head(1, 1)
        att_head(1, 2)
        att_head(1, 3)
        fin_pass_b(0)
        att_head(1, 4)
        att_head(1, 5)
        att_head(1, 6)
        fin_pass_a(1)
        att_head(1, 7)
        # warm-keeper: PE heartbeat through the last norm chain so the
        # trailing kc3 matmuls run at 2.4GHz
        for i in range(40):
            wk = ps_mm.tile([64, 64], F32, tag="mm", name=f"wk{i}")
            nc.tensor.matmul(wk, warm_src[:, 0:64], warm_src[:, 0:64],
                             start=True, stop=True, skip_group_check=True)
        fin_pass_b(1)

    nc.compile()
    return nc


def kernel(x, ln_w, ln_b, w_qkv, w_out):
    x = np.asarray(x, dtype=np.float32)
    ln_w = np.asarray(ln_w, dtype=np.float32)
    ln_b = np.asarray(ln_b, dtype=np.float32)
    w_qkv = np.asarray(w_qkv, dtype=np.float32)
    w_out = np.asarray(w_out, dtype=np.float32)

    # host-side weight folding (LN affine into QKV weights)
    w1 = (w_qkv * ln_w[None, :]).T.astype(BF)            # [D, E]
    b1 = (w_qkv @ ln_b).astype(np.float32)               # [E]
    w2 = w_out.T.astype(BF)                              # [INNER, D]
    use_b1 = bool(np.any(b1))

    # column permutation: [q_c0, k_c0, v, q_c123, k_c567]
    perm = np.concatenate([
        np.arange(0, 128),                  # q chunk 0
        np.arange(512, 640),                # k chunk 0
        np.arange(1024, 1536),              # v
        np.arange(128, 512),                # q chunks 1-3
        np.arange(640, 1024),               # k chunks 5-7
    ])
    w1p = np.ascontiguousarray(w1[:, perm])
    b1p = b1[perm]

    nc = build_graph(use_b1)

    xg = x.reshape(B * P, N, D)
    in_maps = []
    for core in range(N_CORES):
        m = {
            "x": np.ascontiguousarray(xg[G_PER_CORE * core:G_PER_CORE * (core + 1)]),
            "w1": w1p,
            "w2": w2,
        }
        if use_b1:
            m["b1"] = b1p.reshape(E // 128, 128).T.astype(np.float32).copy()
            m["b1v"] = b1p[V_COL:V_COL + INNER].reshape(1, INNER).astype(BF)
        in_maps.append(m)

    trace = bool(int(os.environ.get("KERNEL_TRACE", "0")))
    if trace:
        try:
            import ntff_shim
            ntff_shim.install()
        except Exception as e:
            print(f"ntff shim unavailable: {e}")
            trace = False
    res = run_bass_kernel_spmd(nc, in_maps, list(range(N_CORES)), trace=trace,
                               tmpdir=os.environ.get("KERNEL_TRACE_DIR"))
    global _last_res
    _last_res = res
    if res.exec_time_ns is not None:
        print(f"HW exec time: {res.exec_time_ns} ns")
    out = np.concatenate([r["out"] for r in res.results], axis=0)
    return out.reshape(B, P, N, D)
